# revision 16
# baseline (speedup 1.0000x reference)
"""Multi-head attention (B=4, N=2048, DIM=1024, H=16, HD=64) on 8 TRN2 cores.

Sharding: tensor-parallel over heads - 2 heads per core. The reference omits
the output projection, so each core's output is a disjoint 128-column slice of
the final [B, N, 1024]; no collectives are needed.

Per-core schedule (v5): the ScalarE exp stream (33.5M elems/core, ~1.05us per
[128,1024] chunk) is the roofline; everything else is arranged to keep ACT fed
from the first microseconds to the last:
  - scores^T per (b, qt-wave, kt): two head-packed matmuls (tile_position row
    split) into a double-buffered [128,1024] fp32 PSUM tile; ACT exp -> bf16.
  - av (out^T = [1|v]^T @ expT, denominator in row 0) chases the exp stream
    with a 2-chunk lag instead of bunching at wave end.
  - projection runs as deadline-paced filler at single-matmul granularity (a
    whole 8-matmul group in the in-order PE queue would starve ACT for ~2us).
    Chains are emitted strictly sequentially from one global need-ordered
    list, so the 2-buffer proj PSUM ring is never clobbered mid-chain.
  - x is pre-tiled on the host so each x-tile load is one contiguous 1MB DMA.
  - tail normalize: DVE reciprocal + multiply + bias-add, GpSimd
    partition-broadcast, DMA out.
"""

import numpy as np
import ml_dtypes

import concourse.bacc as bacc
import concourse.mybir as mybir
from concourse.bass_utils import run_bass_kernel_spmd
from concourse.tile import TileContext

B, N, DIM, H = 4, 2048, 1024, 16
HD = DIM // H
SCALE = 1.0 / np.sqrt(HD)
TOK = B * N               # 8192 tokens
NCORES = 8
HPC = H // NCORES         # heads per core = 2

BF16 = mybir.dt.bfloat16
F32 = mybir.dt.float32
AF = mybir.ActivationFunctionType

KT = 8                    # 1024 / 128 contraction tiles
NTB = 4                   # token tiles of 512 per batch
NT = B * NTB              # 16 token tiles total
QT = N // 512             # 4 q-waves per batch
KTOK_B = 16               # k-token tiles of 128 per batch
VROW = 2 * (HD + 1)       # 130: [1 | vA | 1 | vB] per 128-token tile
LAG = 2                   # av trails exp by LAG chunks

COST = {"load": 0, "qkmm": 230, "vmm": 160}
CREDIT_PER_SLOT = 1050 - 222 - 426   # ACT chunk - sc slot - 2 av matmuls


def build_graph():
    nc = bacc.Bacc("TRN2", target_bir_lowering=False, debug=False)
    xtt = nc.declare_dram_parameter("xtt", [NT, 128, KT * 512], BF16,
                                    isOutput=False)
    # wall: pre-tiled [128, kt*256 (qk) | kt*128 (v)] = exact SBUF layout
    wall = nc.declare_dram_parameter("wall", [128, KT * 256 + KT * 128], BF16,
                                     isOutput=False)
    # ball: [128, 4] f32 = [bq | bk | bvqA | bvqB] (bvq on partitions 0-64)
    ball = nc.declare_dram_parameter("ball", [128, 4], F32, isOutput=False)
    out = nc.declare_dram_parameter("out", [HPC, B, HD, N], F32, isOutput=True)

    with TileContext(nc) as tc:
        with (
            tc.tile_pool(name="const", bufs=1) as constp,
            tc.tile_pool(name="qk", bufs=1) as qkp,
            tc.tile_pool(name="xin", bufs=4) as xinp,
            tc.tile_pool(name="exps", bufs=16) as expp,
            tc.tile_pool(name="outs", bufs=4) as outp,
            tc.tile_pool(name="rcs", bufs=4) as rcp,
            tc.tile_pool(name="bcs", bufs=4) as bcp,
        ):
            # warm the ACT exp table set off the critical path
            zz = constp.tile([1, 8], F32)
            zz2 = constp.tile([1, 8], F32)
            nc.vector.memset(zz[:, :], 0.0)
            nc.scalar.activation(zz2[:, :], zz[:, :], AF.Exp)

            # weights: one contiguous host-pretiled DMA, first on the sync
            # queue (it gates the first projection; the sync hwdge queue
            # starts faster than the ACT one)
            wall_s = constp.tile([128, KT * 256 + KT * 128], BF16)
            nc.sync.dma_start(out=wall_s[:, :], in_=wall[:, :])
            wqk_s = wall_s[:, 0:KT * 256]
            wv_s = wall_s[:, KT * 256:]
            ball_s = constp.tile([128, 4], F32)
            nc.sync.dma_start(out=ball_s[:, :], in_=ball[:, :])
            bqk_s = ball_s[:, 0:2]
            bvq_s = ball_s[0:HD + 1, 2:4]

            # PE warmup: dummy matmuls on memset data (no DMA dependency) so
            # the PE p-state is at full clock when real projection arrives
            warm = constp.tile([128, 512], BF16)
            nc.vector.memset(warm[:, :], 0.0)

            q_sb = [qkp.tile([128, N], BF16, name=f"q_sb{_b}") for _b in range(B)]
            k_sb = [qkp.tile([128, N], BF16, name=f"k_sb{_b}") for _b in range(B)]
            v_sb = [qkp.tile([128, KTOK_B * VROW], BF16, name=f"v_sb{_b}")
                    for _b in range(B)]
            for _b in range(B):
                nc.gpsimd.memset(v_sb[_b][:, :], 1.0)

            with (
                tc.tile_pool(name="scps", bufs=2, space="PSUM") as scps,
                tc.tile_pool(name="avps", bufs=1, space="PSUM") as avps,
                tc.tile_pool(name="pjps", bufs=2, space="PSUM") as pjps,
            ):
                # PE p-state warmup chain (no input dependencies)
                wps = pjps.tile([128, 512], F32, name="pj", tag="pj")
                for wi in range(10):
                    nc.tensor.matmul(
                        wps[:, :], lhsT=warm[:, 0:128], rhs=warm[:, :],
                        start=(wi == 0), stop=(wi == 9),
                        skip_group_check=True)

                xnt_tiles = {}
                pj_open = {}

                def emit_item(seg):
                    kind = seg[0]
                    if kind == "load":
                        nt = seg[1]
                        xnt = xinp.tile([128, KT * 512], BF16, name="xnt")
                        nc.sync.dma_start(out=xnt[:, :], in_=xtt[nt])
                        xnt_tiles[nt] = xnt
                    elif kind == "qkmm":
                        _, nt, mt, kt = seg
                        bb, ntb = nt // NTB, nt % NTB
                        if kt == 0:
                            assert not pj_open, f"open chain at {seg}: {pj_open}"
                            pj_open[(nt, mt)] = pjps.tile(
                                [128, 512], F32, name="pj", tag="pj")
                        pj = pj_open[(nt, mt)]
                        nc.tensor.matmul(
                            pj[:, :],
                            lhsT=wqk_s[:, kt * 256 + mt * 128:
                                       kt * 256 + (mt + 1) * 128],
                            rhs=xnt_tiles[nt][:, kt * 512:(kt + 1) * 512],
                            start=(kt == 0), stop=(kt == KT - 1),
                            skip_group_check=True)
                        if kt == KT - 1:
                            dst = q_sb[bb] if mt == 0 else k_sb[bb]
                            nc.vector.tensor_scalar_add(
                                dst[:, ntb * 512:(ntb + 1) * 512], pj[:, :],
                                bqk_s[:, mt:mt + 1])
                            del pj_open[(nt, mt)]
                    elif kind == "vmm":
                        _, nt, sub, kt = seg
                        bb, ntb = nt // NTB, nt % NTB
                        if kt == 0:
                            assert not pj_open, f"open chain at {seg}: {pj_open}"
                            pj_open[(nt, 2, sub)] = pjps.tile(
                                [128, 512], F32, name="pj", tag="pj")
                        pj = pj_open[(nt, 2, sub)]
                        nc.tensor.matmul(
                            pj[:, 0:128],
                            lhsT=xnt_tiles[nt][:, kt * 512 + sub * 128:
                                               kt * 512 + (sub + 1) * 128],
                            rhs=wv_s[:, kt * 128:(kt + 1) * 128],
                            start=(kt == 0), stop=(kt == KT - 1),
                            skip_group_check=True)
                        if kt == KT - 1:
                            tt = ntb * 4 + sub
                            nc.vector.tensor_copy(
                                v_sb[bb][:, tt * VROW + 1: tt * VROW + 1 + HD],
                                pj[:, 0:HD])
                            nc.vector.tensor_copy(
                                v_sb[bb][:, tt * VROW + HD + 2:
                                         tt * VROW + 2 + 2 * HD],
                                pj[:, HD:2 * HD])
                            del pj_open[(nt, 2, sub)]

                # ---- filler: one global need-ordered list of atomic chains,
                # emitted strictly in order at single-matmul grain ----
                def slot(w, kt):
                    return w * 16 + kt

                chains = []   # (need_slot, seq_tiebreak, [items...])
                seqno = 0
                for bb in range(B):
                    w0 = bb * QT
                    sh = 0 if bb == 0 else 16
                    for ntb in range(NTB):
                        nt = bb * NTB + ntb
                        s0 = slot(w0, 4 * ntb)
                        # batch-0 loads 1-3 deferred: only weights+load(0)
                        # gate the first scores; eager loads steal HBM BW
                        ld_dl = (s0 - 4) if (bb == 0 and ntb > 0) else s0 - 6 - sh
                        chains.append((ld_dl, seqno, [("load", nt)]))
                        seqno += 1
                        if ntb == 0:
                            chains.append((s0 - sh, seqno,
                                           [("qkmm", nt, 0, kt)
                                            for kt in range(KT)]))
                            seqno += 1
                        chains.append((s0 - sh, seqno,
                                       [("qkmm", nt, 1, kt)
                                        for kt in range(KT)]))
                        seqno += 1
                        for sub in range(4):
                            chains.append((slot(w0, 4 * ntb + sub) + 1 - sh,
                                           seqno,
                                           [("vmm", nt, sub, kt)
                                            for kt in range(KT)]))
                            seqno += 1
                        if ntb >= 1:
                            chains.append((slot(w0 + ntb, 0) - sh, seqno,
                                           [("qkmm", nt, 0, kt)
                                            for kt in range(KT)]))
                            seqno += 1
                chains.sort(key=lambda c: (c[0], c[1]))
                filler = []
                prev_dl = -10**9
                for need, _, items in chains:
                    n = len(items)
                    for i, it in enumerate(items):
                        dl = need - (n - 1 - i) // 2 - 1
                        dl = max(dl, prev_dl)
                        prev_dl = dl
                        filler.append((dl, it))
                from collections import deque
                filler = deque(filler)
                credit = 0.0

                def pop_filler(sidx):
                    nonlocal credit
                    while filler and filler[0][0] <= sidx:
                        dl, seg = filler.popleft()
                        emit_item(seg)
                        if dl >= 0:
                            credit -= COST[seg[0]]
                    while filler and credit >= COST[filler[0][1][0]]:
                        _, seg = filler.popleft()
                        emit_item(seg)
                        credit -= COST[seg[0]]

                def emit_av(b, pav, e, kt, h):
                    nc.tensor.matmul(
                        pav[h][:, :],
                        lhsT=v_sb[b][:, kt * VROW + h * (HD + 1):
                                     kt * VROW + (h + 1) * (HD + 1)],
                        rhs=e[:, h * 512:(h + 1) * 512],
                        start=(kt == 0), stop=(kt == KTOK_B - 1),
                        skip_group_check=True)

                def emit_tail(b, qt, pav, h):
                    # evacuate pav to SBUF first: frees the PSUM bank for the
                    # next wave's av chain; normalize runs off-critical
                    otc = outp.tile([65, 512], F32, name="otc", tag="otc")
                    nc.vector.tensor_copy(otc[0:65, :], pav[h][0:65, :])
                    rc = rcp.tile([1, 512], F32, name="rc", tag="rc")
                    nc.vector.reciprocal_approx_fast(rc[0:1, :], otc[0:1, :])
                    bcs = bcp.tile([65, 512], F32, name="bcs", tag="bcs")
                    nc.gpsimd.partition_broadcast(bcs[:, :], rc[0:1, :])
                    ot = outp.tile([65, 512], F32, name="ot", tag="ot")
                    nc.vector.tensor_mul(ot[0:65, :], otc[0:65, :],
                                         bcs[0:65, :])
                    ot2 = outp.tile([65, 512], F32, name="ot2", tag="ot2")
                    nc.vector.tensor_scalar_add(ot2[0:65, :], ot[0:65, :],
                                                bvq_s[:, h:h + 1])
                    nc.sync.dma_start(
                        out=out[h, b, :, qt * 512:(qt + 1) * 512],
                        in_=ot2[1:65, :])

                # flat slot pipeline: sc/exp at slot g, avs at g+LAG (tails
                # right after a wave's last avs) — no wave-boundary bunching
                NW = B * QT
                pav_w = {}
                echunk = {}
                for g in range(NW * 16 + LAG):
                    if g < NW * 16:
                        w, kt = g // 16, g % 16
                        b, qt = w // QT, w % QT
                        qcol = qt * 512
                        pop_filler(g)
                        s = scps.tile([128, 1024], F32, name="s", tag="s")
                        for h in range(2):
                            nc.tensor.matmul(
                                s[:, h * 512:(h + 1) * 512],
                                lhsT=k_sb[b][h * 64:(h + 1) * 64,
                                             kt * 128:(kt + 1) * 128],
                                rhs=q_sb[b][h * 64:(h + 1) * 64,
                                            qcol:qcol + 512],
                                start=True, stop=True,
                                tile_position=(h * 64, 0),
                                skip_group_check=True)
                        e = expp.tile([128, 1024], BF16, name="e", tag="e")
                        nc.scalar.activation(e[:, :], s[:, :], AF.Exp)
                        echunk[g] = e
                        credit += CREDIT_PER_SLOT
                    c = g - LAG
                    if c >= 0:
                        wc, ktc = c // 16, c % 16
                        bc, qtc = wc // QT, wc % QT
                        if ktc == 0:
                            pav_w[wc] = [avps.tile([65, 512], F32,
                                                   name=f"pav{_h}",
                                                   tag=f"pav{_h}", bufs=1)
                                         for _h in range(2)]
                        for h in range(2):
                            emit_av(bc, pav_w[wc], echunk[c], ktc, h)
                        del echunk[c]
                        if ktc == KTOK_B - 1:
                            for h in range(2):
                                emit_tail(bc, qtc, pav_w[wc], h)
                            del pav_w[wc]
    nc.compile()
    return nc


_GRAPH = None


def _get_graph():
    global _GRAPH
    if _GRAPH is None:
        _GRAPH = build_graph()
    return _GRAPH


def _make_in_maps(x, w_qkv, b_qkv):
    bf = ml_dtypes.bfloat16
    xT = np.ascontiguousarray(x.reshape(TOK, DIM).T).astype(bf)  # [DIM, TOK]
    # xtt[nt, p, kt*512 + j] = xT[kt*128 + p, nt*512 + j]
    xtt = np.ascontiguousarray(
        xT.reshape(KT, 128, NT, 512).transpose(2, 1, 0, 3).reshape(
            NT, 128, KT * 512))
    in_maps = []
    for c in range(NCORES):
        hA, hB = HPC * c, HPC * c + 1
        rq = [w_qkv[h * HD:(h + 1) * HD] * SCALE for h in (hA, hB)]
        rk = [w_qkv[DIM + h * HD: DIM + (h + 1) * HD] for h in (hA, hB)]
        rv = [w_qkv[2 * DIM + h * HD: 2 * DIM + (h + 1) * HD] for h in (hA, hB)]
        wqk_c = np.concatenate(rq + rk, axis=0).T.astype(np.float32)  # [1024,256]
        wv_c = np.concatenate(rv, axis=0).T.astype(np.float32)       # [1024,128]
        # wall[p, kt*256+j] = wqk_c[kt*128+p, j]; then the same for wv
        wall = np.concatenate([
            wqk_c.reshape(KT, 128, 2 * HPC * HD).transpose(1, 0, 2).reshape(
                128, KT * 2 * HPC * HD),
            wv_c.reshape(KT, 128, HPC * HD).transpose(1, 0, 2).reshape(
                128, KT * HPC * HD)], axis=1)
        wall = np.ascontiguousarray(wall).astype(bf)
        bq = [b_qkv[h * HD:(h + 1) * HD] * SCALE for h in (hA, hB)]
        bk = [b_qkv[DIM + h * HD: DIM + (h + 1) * HD] for h in (hA, hB)]
        bvc = [b_qkv[2 * DIM + h * HD: 2 * DIM + (h + 1) * HD] for h in (hA, hB)]
        ball = np.zeros((128, 4), dtype=np.float32)
        ball[:, 0] = np.concatenate(bq)
        ball[:, 1] = np.concatenate(bk)
        for hh in range(HPC):
            ball[1:HD + 1, 2 + hh] = bvc[hh]
        in_maps.append({"xtt": xtt, "wall": wall, "ball": ball})
    return in_maps


def _run(x, w_qkv, b_qkv, trace=False, tmpdir=None):
    nc = _get_graph()
    in_maps = _make_in_maps(np.asarray(x, dtype=np.float32),
                            np.asarray(w_qkv, dtype=np.float32),
                            np.asarray(b_qkv, dtype=np.float32))
    res = run_bass_kernel_spmd(nc, in_maps, core_ids=list(range(NCORES)),
                               trace=trace, tmpdir=tmpdir)
    full = np.empty((B, N, DIM), dtype=np.float32)
    for c in range(NCORES):
        oc = res.results[c]["out"]          # [HPC, B, HD, N]
        full[:, :, c * HPC * HD:(c + 1) * HPC * HD] = \
            oc.transpose(1, 3, 0, 2).reshape(B, N, HPC * HD)
    return full, res


def kernel(x, w_qkv, b_qkv):
    full, _ = _run(x, w_qkv, b_qkv, trace=False)
    return full


# revision 19
# speedup vs baseline: 1.1464x; 1.1464x over previous
"""Multi-head attention (B=4, N=2048, DIM=1024, H=16, HD=64) on 8 TRN2 cores.

Sharding: tensor-parallel over heads - 2 heads per core. The reference omits
the output projection, so each core's output is a disjoint 128-column slice of
the final [B, N, 1024]; no collectives are needed.

Per-core schedule (v5): the ScalarE exp stream (33.5M elems/core, ~1.05us per
[128,1024] chunk) is the roofline; everything else is arranged to keep ACT fed
from the first microseconds to the last:
  - scores^T per (b, qt-wave, kt): two head-packed matmuls (tile_position row
    split) into a double-buffered [128,1024] fp32 PSUM tile; ACT exp -> bf16.
  - av (out^T = [1|v]^T @ expT, denominator in row 0) chases the exp stream
    with a 2-chunk lag instead of bunching at wave end.
  - projection runs as deadline-paced filler at single-matmul granularity (a
    whole 8-matmul group in the in-order PE queue would starve ACT for ~2us).
    Chains are emitted strictly sequentially from one global need-ordered
    list, so the 2-buffer proj PSUM ring is never clobbered mid-chain.
  - x is pre-tiled on the host so each x-tile load is one contiguous 1MB DMA.
  - tail normalize: DVE reciprocal + multiply + bias-add, GpSimd
    partition-broadcast, DMA out.
"""

import numpy as np
import ml_dtypes

import concourse.bacc as bacc
import concourse.mybir as mybir
from concourse.bass_utils import run_bass_kernel_spmd
from concourse.tile import TileContext

B, N, DIM, H = 4, 2048, 1024, 16
HD = DIM // H
SCALE = 1.0 / np.sqrt(HD)
TOK = B * N               # 8192 tokens
NCORES = 8
HPC = H // NCORES         # heads per core = 2

BF16 = mybir.dt.bfloat16
F32 = mybir.dt.float32
AF = mybir.ActivationFunctionType

KT = 8                    # 1024 / 128 contraction tiles
NTB = 4                   # token tiles of 512 per batch
NT = B * NTB              # 16 token tiles total
QT = N // 512             # 4 q-waves per batch
KTOK_B = 16               # k-token tiles of 128 per batch
VROW = 2 * (HD + 1)       # 130: [1 | vA | 1 | vB] per 128-token tile
LAG = 2                   # av trails exp by LAG chunks

COST = {"load": 0, "qkmm": 230, "vmm": 160}
CREDIT_PER_SLOT = 1050 - 222 - 426   # ACT chunk - sc slot - 2 av matmuls


def build_graph():
    nc = bacc.Bacc("TRN2", target_bir_lowering=False, debug=False)
    xtt = nc.declare_dram_parameter("xtt", [NT, 128, KT * 512], BF16,
                                    isOutput=False)
    # wall: pre-tiled [128, kt*256 (qk) | kt*128 (v)] = exact SBUF layout
    wall = nc.declare_dram_parameter("wall", [128, KT * 256 + KT * 128], BF16,
                                     isOutput=False)
    # ball: [128, 4] f32 = [bq | bk | bvqA | bvqB] (bvq on partitions 0-64)
    ball = nc.declare_dram_parameter("ball", [128, 4], F32, isOutput=False)
    out = nc.declare_dram_parameter("out", [HPC, B, HD, N], F32, isOutput=True)

    with TileContext(nc) as tc:
        with (
            tc.tile_pool(name="const", bufs=1) as constp,
            tc.tile_pool(name="qk", bufs=1) as qkp,
            tc.tile_pool(name="xin", bufs=4) as xinp,
            tc.tile_pool(name="exps", bufs=16) as expp,
            tc.tile_pool(name="outs", bufs=4) as outp,
            tc.tile_pool(name="rcs", bufs=4) as rcp,
            tc.tile_pool(name="bcs", bufs=4) as bcp,
        ):
            # warm the ACT exp table set off the critical path
            zz = constp.tile([1, 8], F32)
            zz2 = constp.tile([1, 8], F32)
            nc.vector.memset(zz[:, :], 0.0)
            nc.scalar.activation(zz2[:, :], zz[:, :], AF.Exp)

            # weights: one contiguous host-pretiled DMA, first on the sync
            # queue (it gates the first projection; the sync hwdge queue
            # starts faster than the ACT one)
            wall_s = constp.tile([128, KT * 256 + KT * 128], BF16)
            nc.sync.dma_start(out=wall_s[:, :], in_=wall[:, :])
            wqk_s = wall_s[:, 0:KT * 256]
            wv_s = wall_s[:, KT * 256:]
            ball_s = constp.tile([128, 4], F32)
            nc.sync.dma_start(out=ball_s[:, :], in_=ball[:, :])
            bqk_s = ball_s[:, 0:2]
            bvq_s = ball_s[0:HD + 1, 2:4]

            # PE warmup: dummy matmuls on memset data (no DMA dependency) so
            # the PE p-state is at full clock when real projection arrives
            warm = constp.tile([128, 512], BF16)
            nc.vector.memset(warm[:, :], 0.0)

            q_sb = [qkp.tile([128, N], BF16, name=f"q_sb{_b}") for _b in range(B)]
            k_sb = [qkp.tile([128, N], BF16, name=f"k_sb{_b}") for _b in range(B)]
            v_sb = [qkp.tile([128, KTOK_B * VROW], BF16, name=f"v_sb{_b}")
                    for _b in range(B)]
            for _b in range(B):
                nc.gpsimd.memset(v_sb[_b][:, :], 1.0)

            with (
                tc.tile_pool(name="scps", bufs=2, space="PSUM") as scps,
                tc.tile_pool(name="avps", bufs=1, space="PSUM") as avps,
                tc.tile_pool(name="pjps", bufs=2, space="PSUM") as pjps,
            ):
                # PE p-state warmup chain (no input dependencies)
                wps = pjps.tile([128, 512], F32, name="pj", tag="pj")
                for wi in range(10):
                    nc.tensor.matmul(
                        wps[:, :], lhsT=warm[:, 0:128], rhs=warm[:, :],
                        start=(wi == 0), stop=(wi == 9),
                        skip_group_check=True)

                xnt_tiles = {}
                pj_open = {}

                def emit_item(seg):
                    kind = seg[0]
                    if kind == "load":
                        nt = seg[1]
                        xnt = xinp.tile([128, KT * 512], BF16, name="xnt")
                        nc.sync.dma_start(out=xnt[:, :], in_=xtt[nt])
                        xnt_tiles[nt] = xnt
                    elif kind == "qkmm":
                        _, nt, mt, kt = seg
                        bb, ntb = nt // NTB, nt % NTB
                        if kt == 0:
                            assert not pj_open, f"open chain at {seg}: {pj_open}"
                            pj_open[(nt, mt)] = pjps.tile(
                                [128, 512], F32, name="pj", tag="pj")
                        pj = pj_open[(nt, mt)]
                        nc.tensor.matmul(
                            pj[:, :],
                            lhsT=wqk_s[:, kt * 256 + mt * 128:
                                       kt * 256 + (mt + 1) * 128],
                            rhs=xnt_tiles[nt][:, kt * 512:(kt + 1) * 512],
                            start=(kt == 0), stop=(kt == KT - 1),
                            skip_group_check=True)
                        if kt == KT - 1:
                            dst = q_sb[bb] if mt == 0 else k_sb[bb]
                            nc.vector.tensor_scalar_add(
                                dst[:, ntb * 512:(ntb + 1) * 512], pj[:, :],
                                bqk_s[:, mt:mt + 1])
                            del pj_open[(nt, mt)]
                    elif kind == "vmm":
                        _, nt, sub, kt = seg
                        bb, ntb = nt // NTB, nt % NTB
                        if kt == 0:
                            assert not pj_open, f"open chain at {seg}: {pj_open}"
                            pj_open[(nt, 2, sub)] = pjps.tile(
                                [128, 512], F32, name="pj", tag="pj")
                        pj = pj_open[(nt, 2, sub)]
                        nc.tensor.matmul(
                            pj[:, 0:128],
                            lhsT=xnt_tiles[nt][:, kt * 512 + sub * 128:
                                               kt * 512 + (sub + 1) * 128],
                            rhs=wv_s[:, kt * 128:(kt + 1) * 128],
                            start=(kt == 0), stop=(kt == KT - 1),
                            skip_group_check=True)
                        if kt == KT - 1:
                            tt = ntb * 4 + sub
                            nc.vector.tensor_copy(
                                v_sb[bb][:, tt * VROW + 1: tt * VROW + 1 + HD],
                                pj[:, 0:HD])
                            nc.vector.tensor_copy(
                                v_sb[bb][:, tt * VROW + HD + 2:
                                         tt * VROW + 2 + 2 * HD],
                                pj[:, HD:2 * HD])
                            del pj_open[(nt, 2, sub)]

                # ---- filler: one global need-ordered list of atomic chains,
                # emitted strictly in order at single-matmul grain ----
                def slot(w, kt):
                    return w * 16 + kt

                chains = []   # (need_slot, seq_tiebreak, [items...])
                seqno = 0
                for bb in range(B):
                    w0 = bb * QT
                    sh = 0 if bb == 0 else 16
                    for ntb in range(NTB):
                        nt = bb * NTB + ntb
                        s0 = slot(w0, 4 * ntb)
                        # batch-0 loads 1-3 deferred: only weights+load(0)
                        # gate the first scores; eager loads steal HBM BW
                        ld_dl = (s0 - 4) if (bb == 0 and ntb > 0) else s0 - 6 - sh
                        chains.append((ld_dl, seqno, [("load", nt)]))
                        seqno += 1
                        if ntb == 0:
                            chains.append((s0 - sh, seqno,
                                           [("qkmm", nt, 0, kt)
                                            for kt in range(KT)]))
                            seqno += 1
                        chains.append((s0 - sh, seqno,
                                       [("qkmm", nt, 1, kt)
                                        for kt in range(KT)]))
                        seqno += 1
                        for sub in range(4):
                            chains.append((slot(w0, 4 * ntb + sub) + 1 - sh,
                                           seqno,
                                           [("vmm", nt, sub, kt)
                                            for kt in range(KT)]))
                            seqno += 1
                        if ntb >= 1:
                            chains.append((slot(w0 + ntb, 0) - sh, seqno,
                                           [("qkmm", nt, 0, kt)
                                            for kt in range(KT)]))
                            seqno += 1
                chains.sort(key=lambda c: (c[0], c[1]))
                filler = []
                prev_dl = -10**9
                for need, _, items in chains:
                    n = len(items)
                    for i, it in enumerate(items):
                        dl = need - (n - 1 - i) // 2 - 1
                        dl = max(dl, prev_dl)
                        prev_dl = dl
                        filler.append((dl, it))
                from collections import deque
                filler = deque(filler)
                credit = 0.0

                def pop_filler(sidx):
                    nonlocal credit
                    while filler and filler[0][0] <= sidx:
                        dl, seg = filler.popleft()
                        emit_item(seg)
                        if dl >= 0:
                            credit -= COST[seg[0]]
                    while filler and credit >= COST[filler[0][1][0]]:
                        _, seg = filler.popleft()
                        emit_item(seg)
                        credit -= COST[seg[0]]

                def emit_av(b, pav, e, kt, h):
                    nc.tensor.matmul(
                        pav[h][:, :],
                        lhsT=v_sb[b][:, kt * VROW + h * (HD + 1):
                                     kt * VROW + (h + 1) * (HD + 1)],
                        rhs=e[:, h * 512:(h + 1) * 512],
                        start=(kt == 0), stop=(kt == KTOK_B - 1),
                        skip_group_check=True)

                def emit_tail(b, qt, pav, h):
                    dn = rcp.tile([1, 512], F32, name="dn", tag="dn")
                    nc.vector.tensor_copy(dn[0:1, :], pav[h][0:1, :])
                    rc = rcp.tile([1, 512], F32, name="rc", tag="rc")
                    nc.vector.reciprocal_approx_fast(rc[0:1, :], dn[0:1, :])
                    bcs = bcp.tile([65, 512], F32, name="bcs", tag="bcs")
                    nc.gpsimd.partition_broadcast(bcs[:, :], rc[0:1, :])
                    ot = outp.tile([65, 512], F32, name="ot", tag="ot")
                    nc.vector.tensor_mul(ot[0:65, :], pav[h][0:65, :],
                                         bcs[0:65, :])
                    ot2 = outp.tile([65, 512], F32, name="ot2", tag="ot2")
                    nc.vector.tensor_scalar_add(ot2[0:65, :], ot[0:65, :],
                                                bvq_s[:, h:h + 1])
                    nc.sync.dma_start(
                        out=out[h, b, :, qt * 512:(qt + 1) * 512],
                        in_=ot2[1:65, :])

                # flat slot pipeline: sc/exp at slot g; avs trail per AVOFF
                # (first chunks of each wave delayed so the previous wave's
                # tail has released the pav banks - avoids an in-order PE
                # stall that starves ACT at every wave boundary)
                AVOFF = [5, 5, 6, 6, 7] + [j + 2 for j in range(5, 16)]
                NW = B * QT
                pav_w = {}
                echunk = {}
                pending = []
                for g in range(NW * 16 + AVOFF[-1] - 15 + 1):
                    if g < NW * 16:
                        w, kt = g // 16, g % 16
                        b, qt = w // QT, w % QT
                        qcol = qt * 512
                        pop_filler(g)
                        s = scps.tile([128, 1024], F32, name="s", tag="s")
                        for h in range(2):
                            nc.tensor.matmul(
                                s[:, h * 512:(h + 1) * 512],
                                lhsT=k_sb[b][h * 64:(h + 1) * 64,
                                             kt * 128:(kt + 1) * 128],
                                rhs=q_sb[b][h * 64:(h + 1) * 64,
                                            qcol:qcol + 512],
                                start=True, stop=True,
                                tile_position=(h * 64, 0),
                                skip_group_check=True)
                        e = expp.tile([128, 1024], BF16, name="e", tag="e")
                        nc.scalar.activation(e[:, :], s[:, :], AF.Exp)
                        echunk[g] = e
                        pending.append(g)
                        credit += CREDIT_PER_SLOT
                    while pending and \
                            (pending[0] // 16) * 16 + AVOFF[pending[0] % 16] <= g:
                        c = pending.pop(0)
                        wc, ktc = c // 16, c % 16
                        bc, qtc = wc // QT, wc % QT
                        if ktc == 0:
                            pav_w[wc] = [avps.tile([65, 512], F32,
                                                   name=f"pav{_h}",
                                                   tag=f"pav{_h}", bufs=1)
                                         for _h in range(2)]
                        for h in range(2):
                            emit_av(bc, pav_w[wc], echunk[c], ktc, h)
                        del echunk[c]
                        if ktc == KTOK_B - 1:
                            for h in range(2):
                                emit_tail(bc, qtc, pav_w[wc], h)
                            del pav_w[wc]
    nc.compile()
    return nc


_GRAPH = None


def _get_graph():
    global _GRAPH
    if _GRAPH is None:
        _GRAPH = build_graph()
    return _GRAPH


def _make_in_maps(x, w_qkv, b_qkv):
    bf = ml_dtypes.bfloat16
    xT = np.ascontiguousarray(x.reshape(TOK, DIM).T).astype(bf)  # [DIM, TOK]
    # xtt[nt, p, kt*512 + j] = xT[kt*128 + p, nt*512 + j]
    xtt = np.ascontiguousarray(
        xT.reshape(KT, 128, NT, 512).transpose(2, 1, 0, 3).reshape(
            NT, 128, KT * 512))
    in_maps = []
    for c in range(NCORES):
        hA, hB = HPC * c, HPC * c + 1
        rq = [w_qkv[h * HD:(h + 1) * HD] * SCALE for h in (hA, hB)]
        rk = [w_qkv[DIM + h * HD: DIM + (h + 1) * HD] for h in (hA, hB)]
        rv = [w_qkv[2 * DIM + h * HD: 2 * DIM + (h + 1) * HD] for h in (hA, hB)]
        wqk_c = np.concatenate(rq + rk, axis=0).T.astype(np.float32)  # [1024,256]
        wv_c = np.concatenate(rv, axis=0).T.astype(np.float32)       # [1024,128]
        # wall[p, kt*256+j] = wqk_c[kt*128+p, j]; then the same for wv
        wall = np.concatenate([
            wqk_c.reshape(KT, 128, 2 * HPC * HD).transpose(1, 0, 2).reshape(
                128, KT * 2 * HPC * HD),
            wv_c.reshape(KT, 128, HPC * HD).transpose(1, 0, 2).reshape(
                128, KT * HPC * HD)], axis=1)
        wall = np.ascontiguousarray(wall).astype(bf)
        bq = [b_qkv[h * HD:(h + 1) * HD] * SCALE for h in (hA, hB)]
        bk = [b_qkv[DIM + h * HD: DIM + (h + 1) * HD] for h in (hA, hB)]
        bvc = [b_qkv[2 * DIM + h * HD: 2 * DIM + (h + 1) * HD] for h in (hA, hB)]
        ball = np.zeros((128, 4), dtype=np.float32)
        ball[:, 0] = np.concatenate(bq)
        ball[:, 1] = np.concatenate(bk)
        for hh in range(HPC):
            ball[1:HD + 1, 2 + hh] = bvc[hh]
        in_maps.append({"xtt": xtt, "wall": wall, "ball": ball})
    return in_maps


def _run(x, w_qkv, b_qkv, trace=False, tmpdir=None):
    nc = _get_graph()
    in_maps = _make_in_maps(np.asarray(x, dtype=np.float32),
                            np.asarray(w_qkv, dtype=np.float32),
                            np.asarray(b_qkv, dtype=np.float32))
    res = run_bass_kernel_spmd(nc, in_maps, core_ids=list(range(NCORES)),
                               trace=trace, tmpdir=tmpdir)
    full = np.empty((B, N, DIM), dtype=np.float32)
    for c in range(NCORES):
        oc = res.results[c]["out"]          # [HPC, B, HD, N]
        full[:, :, c * HPC * HD:(c + 1) * HPC * HD] = \
            oc.transpose(1, 3, 0, 2).reshape(B, N, HPC * HD)
    return full, res


def kernel(x, w_qkv, b_qkv):
    full, _ = _run(x, w_qkv, b_qkv, trace=False)
    return full
